# revision 31
# baseline (speedup 1.0000x reference)
"""Trainium2 Bass kernel for the binarized 2-layer MLP (nn_FC_small).

Network (reference semantics):
    h  = sign(x) @ sign(W1).T            # [B, 512], B = 65536, in = 768
    h  = batchnorm(h, g1, b1)            # training-mode, full-batch stats
    h  = clip(h, -1, 1)                  # hardtanh (sign-preserving)
    o  = sign(h) @ sign(W2).T            # [B, 10]
    o  = batchnorm(o, g2, b2)

Key identities used:
  * sign(clip(z)) == sign(z), and with g>0:
    sign((h-mu)*r*g + b) == sign(h - T), T = mu - b/(r*g).
  * sign(v) = 2*(v>0) - 1 for v != 0 (exact-zero ties handled by ACT Sign
    where it matters; measured data margin makes them impossible anyway).
  * mm = (x>0) @ sign(W1).T  =>  h1 = 2*mm - c1; threshold/stats are
    affine-transformed so only mm is ever materialized (exact integers).

Sharding: data-parallel over the batch across 8 NeuronCores (8192 rows each).
BN statistics are combined with two tiny AllGathers.

Dataflow per core:
  gpsimd cast-DMA x (fp32->bf16) -> PE-transpose (bf16, ~88ns/tile) ->
  fused code-gen evac from PSUM (DVE is_gt {0,1} / ACT Sign {+-1}, absorbed
  into per-chunk stationary scaling) -> fp8 DoubleRow matmul vs sign(W1).T
  -> exact integer counts in PSUM -> f16 h1-store + bn_stats -> warm
  AllGather of stats -> per-column thresholds -> {0,1} s-codes (DVE) ->
  bf16 matmul vs 2*sign(W2).T -> bn_stats -> AllGather -> affine ->
  PE-transpose to natural [8192, 10] output (host just concatenates).
"""
import numpy as np

import concourse.bass as bass
import concourse.bacc as bacc
import concourse.tile as tile
import concourse.mybir as mybir
from concourse import bass_utils
from concourse.masks import make_identity

F32 = mybir.dt.float32
F16 = mybir.dt.float16
BF16 = mybir.dt.bfloat16
FP8 = mybir.dt.float8e4
GT = mybir.AluOpType.is_gt
MUL = mybir.AluOpType.mult
ADD = mybir.AluOpType.add
SUB = mybir.AluOpType.subtract
DR = mybir.MatmulPerfMode.DoubleRow

N_CORES = 8
B = 65536
IND, HID, OUT = 768, 512, 10
B_LOC = B // N_CORES          # 8192
BC = 512                      # batch chunk
NBC = B_LOC // BC             # 16
KC = IND // 128               # 6 k-chunks
HC = HID // 128               # 4 hid-chunks
EPS = 1e-5

_cache = {}


def build():
    if "nc" in _cache:
        return _cache["nc"]
    nc = bacc.Bacc("TRN2", target_bir_lowering=False, debug=False,
                   num_devices=N_CORES)
    x = nc.dram_tensor("x", [B_LOC, IND], F32, kind="ExternalInput")
    w1 = nc.dram_tensor("w1", [HID, IND], F32, kind="ExternalInput")
    w2 = nc.dram_tensor("w2", [OUT, HID], F32, kind="ExternalInput")
    g1 = nc.dram_tensor("g1", [HID], F32, kind="ExternalInput")
    b1 = nc.dram_tensor("b1", [HID], F32, kind="ExternalInput")
    g2 = nc.dram_tensor("g2", [OUT], F32, kind="ExternalInput")
    b2 = nc.dram_tensor("b2", [OUT], F32, kind="ExternalInput")
    o_out = nc.dram_tensor("o_out", [OUT, B_LOC], F32, kind="ExternalOutput")

    with tile.TileContext(nc) as tc:
        with (
            tc.tile_pool(name="cst", bufs=1) as cst,      # constants / persistents
            tc.tile_pool(name="stage", bufs=3) as stage,  # streaming tiles
            tc.tile_pool(name="psx", bufs=4, space="PSUM") as psx,
            tc.tile_pool(name="psmm", bufs=1, space="PSUM") as psmm,
            tc.tile_pool(name="dram", bufs=1, space="DRAM") as dpool,
        ):
            ident = cst.tile([128, 128], BF16)
            make_identity(nc, ident[:])
            identf16 = cst.tile([16, 16], F32)
            make_identity(nc, identf16[:])
            wup_sb = cst.tile([128, 1], F32)
            bias_m1 = cst.tile([128, 1], F32)
            nc.vector.memset(bias_m1[:], -1.0)
            bias_m2 = cst.tile([128, 1], F32)
            nc.vector.memset(bias_m2[:], -2.0)

            # prefetch the first x chunk ahead of weight prep
            xc_first = stage.tile([128, 4, IND], BF16, tag="xc")
            nc.gpsimd.dma_start(
                out=xc_first[:],
                in_=x.ap()[0:BC].rearrange("(s p) f -> p s f", p=128))

            # ---------------- weight prep (one-time, tiny) ----------------
            # W1 natural [512, 768] -> [128, 4, 768] (partition = hid%128)
            w1c = stage.tile([128, 4, IND], BF16, tag="xc")
            nc.gpsimd.dma_start(out=w1c[:], in_=w1.ap().rearrange("(c p) f -> p c f", p=128))
            w1b = stage.tile([128, 4, IND], BF16, tag="xc")
            nc.vector.tensor_scalar(w1b[:], w1c[:], 0.0, None, GT)
            # transpose each [128 hid, 128 feat] block -> [feat, hid]
            w1sT = cst.tile([128, KC, HID], FP8)     # sign(W1).T in fp8 (+-1)
            for k in range(KC):
                pw = psmm.tile([128, HC, 128], BF16, tag=f"mm{k % 4}")
                for c in range(HC):
                    nc.tensor.transpose(pw[:, c, :], w1b[:, c, k * 128:(k + 1) * 128], ident[:])
                # k<3 pairs with {0,1} x-codes: w values +-2; k>=3 pairs with
                # +-1 x-codes: w values +-1.  (2b-1)*s for s in {1,2}.
                sc1 = 2.0 if k >= 3 else 4.0
                nc.scalar.activation(w1sT[:, k, :], pw[:].rearrange("p c f -> p (c f)"),
                                     mybir.ActivationFunctionType.Identity,
                                     bias=(bias_m1[:] if k >= 3 else bias_m2[:]), scale=sc1)

            # ---------------- persistent big buffers ----------------
            xT8 = cst.tile([128, KC, B_LOC], FP8, tag="big8")   # 48 KB/part
            h1s = cst.tile([128, HC, B_LOC], F16, tag="hbig")   # 64 KB/part
            st1 = cst.tile([128, HC, NBC * 6], F32)        # bn_stats accum
            h2T = cst.tile([OUT, B_LOC], F32, tag="hbig")  # reuses h1s slot
            st2 = cst.tile([OUT, NBC * 6], F32)

            # warm-up collective: pays ncfw cold-start during phase A.
            # high_priority pins it to the schedule start (the CC instruction
            # is only a doorbell; completion is waited on by consumers only).
            wloc = dpool.tile([128, 1], F32)
            wgat = dpool.tile([128 * N_CORES, 1], F32)
            with tc.high_priority():
                nc.vector.memset(wup_sb[:], 0.0)
                nc.sync.dma_start(out=wloc[:], in_=wup_sb[:])
                nc.gpsimd.collective_compute(
                    "AllGather", mybir.AluOpType.bypass,
                    ins=[wloc.opt()], outs=[wgat.opt()],
                    replica_groups=[list(range(N_CORES))])

            # ---------------- phase A: x -> codes -> mm1 -> h1 store ----------------
            BLK = 4
            with nc.named_scope("phaseA"):
                for blk in range(NBC // BLK):
                    for b4 in range(BLK):
                        bc = blk * BLK + b4
                        bs = bc * BC
                        if bc == 0:
                            xc = xc_first
                        else:
                            xc = stage.tile([128, 4, IND], BF16, tag="xc")
                            nc.gpsimd.dma_start(
                                out=xc[:],
                                in_=x.ap()[bs:bs + BC].rearrange("(s p) f -> p s f", p=128))
                        for k in range(KC):
                            pt = psx.tile([128, 4, 128], BF16, tag="pt")
                            for s in range(4):
                                nc.tensor.transpose(pt[:, s, :], xc[:, s, k * 128:(k + 1) * 128], ident[:])
                            ptv = pt[:].rearrange("p s f -> p (s f)")
                            if k < 3:
                                nc.vector.tensor_scalar(xT8[:, k, bs:bs + BC], ptv, 0.0, None, GT)
                            else:
                                nc.scalar.sign(xT8[:, k, bs:bs + BC], ptv)
                    for h in range(HC):
                        mps = []
                        for b4 in range(BLK):
                            bs = (blk * BLK + b4) * BC
                            mp = psmm.tile([128, BC], F32, tag=f"mm{b4}")
                            mps.append(mp)
                            for k2 in range(KC // 2):
                                nc.tensor.matmul(
                                    mp[:],
                                    w1sT[:, 2 * k2:2 * k2 + 2, h * 128:(h + 1) * 128],
                                    xT8[:, 2 * k2:2 * k2 + 2, bs:bs + BC],
                                    start=(k2 == 0), stop=(k2 == KC // 2 - 1),
                                    perf_mode=DR)
                        for b4 in range(BLK):
                            bc = blk * BLK + b4
                            bs = bc * BC
                            nc.scalar.copy(h1s[:, h, bs:bs + BC], mps[b4][:])
                            nc.vector.bn_stats(st1[:, h, bc * 6:(bc + 1) * 6], mps[b4][:])

            # W2 natural [10, 512]
            w2n = cst.tile([OUT, HID], F32)
            nc.sync.dma_start(out=w2n[:], in_=w2.ap())
            w2b = cst.tile([OUT, HID], BF16)
            nc.vector.tensor_scalar(w2b[:], w2n[:], 0.0, None, GT)
            # sign(W2).T as fp8 [128, 4, 10]; hid-chunks 0,1 scaled x1 (for +-1 s),
            # hid-chunks 2,3 scaled x2 (for {0,1} s via is_gt; fold 2x into weight)
            w2sT = cst.tile([128, HC, 16], BF16)
            nc.vector.memset(w2sT[:], 0.0)
            for c in range(HC):
                pw2 = psmm.tile([128, OUT], BF16, tag="mm1")
                nc.tensor.transpose(pw2[:], w2b[:, c * 128:(c + 1) * 128], ident[:OUT, :OUT])
                sc = 4.0
                nc.scalar.activation(w2sT[:, c, 0:OUT], pw2[:],
                                     mybir.ActivationFunctionType.Identity,
                                     bias=bias_m2[:], scale=sc)
            # per-partition copies of g/b vectors
            g1c = cst.tile([128, HC], F32)
            b1c = cst.tile([128, HC], F32)
            for c in range(HC):
                nc.sync.dma_start(out=g1c[:, c:c + 1], in_=g1.ap()[c * 128:(c + 1) * 128])
                nc.sync.dma_start(out=b1c[:, c:c + 1], in_=b1.ap()[c * 128:(c + 1) * 128])
            g2c = cst.tile([OUT, 1], F32)
            b2c = cst.tile([OUT, 1], F32)
            nc.sync.dma_start(out=g2c[:], in_=g2.ap())
            nc.sync.dma_start(out=b2c[:], in_=b2.ap())

            # local aggregate -> [128, 4, 2] (mean, var) of mm per hid col
            agg1 = cst.tile([128, HC, 2], F32)
            for h in range(HC):
                nc.vector.bn_aggr(agg1[:, h, :], st1[:, h, :].rearrange("p (n s) -> p n s", s=6))

            # ---------------- AllGather 1 ----------------
            loc1 = dpool.tile([128, HC * 2], F32)
            gat1 = dpool.tile([128 * N_CORES, HC * 2], F32)
            nc.gpsimd.dma_start(out=loc1[:], in_=agg1[:].rearrange("p c s -> p (c s)"))
            nc.gpsimd.collective_compute(
                "AllGather", mybir.AluOpType.bypass,
                ins=[loc1.opt()], outs=[gat1.opt()],
                replica_groups=[list(range(N_CORES))])
            ga1 = cst.tile([128, N_CORES, HC * 2], F32)
            nc.sync.dma_start(out=ga1[:], in_=gat1[:].rearrange("(c p) s -> p c s", p=128))

            # combine: mean_tot = avg(mean_c); var_tot = avg(var_c + mean_c^2) - mean_tot^2
            with nc.named_scope("combine1"):
                q1 = cst.tile([128, N_CORES, HC * 2], F32)
                nc.vector.tensor_tensor(q1[:], ga1[:], ga1[:], MUL)
                msum = cst.tile([128, HC * 2], F32)
                qsum = cst.tile([128, HC * 2], F32)
                # reduce over the core axis in one op (view [p, s, c], reduce X)
                nc.vector.tensor_reduce(msum[:], ga1[:].rearrange("p c s -> p s c"),
                                        mybir.AxisListType.X, ADD)
                nc.vector.tensor_reduce(qsum[:], q1[:].rearrange("p c s -> p s c"),
                                        mybir.AxisListType.X, ADD)
                # views: even cols = means, odd = vars
                m1 = cst.tile([128, HC], F32)   # global mean of mm
                v1 = cst.tile([128, HC], F32)   # global var of mm
                mview = msum[:].rearrange("p (c s) -> p c s", s=2)
                qview = qsum[:].rearrange("p (c s) -> p c s", s=2)
                nc.vector.tensor_scalar(m1[:], mview[:, :, 0], 1.0 / N_CORES, None, MUL)
                # E[mm^2] = (sum var_c + sum mean_c^2)/8 ; var = E[mm^2] - m1^2
                e2 = cst.tile([128, HC], F32)
                nc.vector.tensor_tensor(e2[:], qview[:, :, 0], mview[:, :, 1], ADD)
                nc.vector.tensor_scalar(e2[:], e2[:], 1.0 / N_CORES, None, MUL)
                m1sq = cst.tile([128, HC], F32)
                nc.vector.tensor_tensor(m1sq[:], m1[:], m1[:], MUL)
                nc.vector.tensor_tensor(v1[:], e2[:], m1sq[:], SUB)
                # h1 = psum - c1_D (unit scale); sd1 = sqrt(v1 + eps) = 1/r1
                sd1 = cst.tile([128, HC], F32)
                nc.vector.tensor_scalar(sd1[:], v1[:], 1.0, EPS, MUL, ADD)
                nc.scalar.sqrt(sd1[:], sd1[:])
                # threshold in psum units: Tm = m1 - b1*sd1/g1
                ig1 = cst.tile([128, HC], F32)
                nc.vector.reciprocal(ig1[:], g1c[:])
                corr = cst.tile([128, HC], F32)
                nc.vector.tensor_tensor(corr[:], b1c[:], ig1[:], MUL)
                nc.vector.tensor_tensor(corr[:], corr[:], sd1[:], MUL)
                posT = cst.tile([128, HC], F32)   # +Tm for is_gt
                negT = cst.tile([128, HC], F32)   # -Tm for ACT Sign bias
                nc.vector.tensor_tensor(posT[:], m1[:], corr[:], SUB)
                nc.vector.tensor_scalar(negT[:], posT[:], -1.0, None, MUL)

            # ---------------- phase B: sign -> mm2 -> h2 ----------------
            with nc.named_scope("phaseB"):
                s8 = cst.tile([128, HC, B_LOC], BF16, tag="bigs")
                SLAB = 2048
                for sl in range(B_LOC // SLAB):
                    ss = sl * SLAB
                    for h in range(HC):
                        nc.vector.tensor_scalar(s8[:, h, ss:ss + SLAB], h1s[:, h, ss:ss + SLAB],
                                                posT[:, h:h + 1], None, GT)
                for bc in range(NBC):
                    bs = bc * BC
                    mp2 = psmm.tile([16, BC], F32, tag=f"mm{bc % 4}")
                    for k in range(HC):
                        nc.tensor.matmul(
                            mp2[:],
                            w2sT[:, k, :],
                            s8[:, k, bs:bs + BC],
                            start=(k == 0), stop=(k == HC - 1))
                    nc.scalar.copy(h2T[:, bs:bs + BC], mp2[:OUT, :])
                    nc.vector.bn_stats(st2[:, bc * 6:(bc + 1) * 6], mp2[:OUT, :])

            agg2 = cst.tile([OUT, 2], F32)
            nc.vector.bn_aggr(agg2[:], st2[:].rearrange("p (n s) -> p n s", s=6))

            # ---------------- AllGather 2 ----------------
            loc2 = dpool.tile([OUT, 2], F32)
            gat2 = dpool.tile([OUT * N_CORES, 2], F32)
            nc.gpsimd.dma_start(out=loc2[:], in_=agg2[:])
            nc.gpsimd.collective_compute(
                "AllGather", mybir.AluOpType.bypass,
                ins=[loc2.opt()], outs=[gat2.opt()],
                replica_groups=[list(range(N_CORES))])
            ga2 = cst.tile([OUT, N_CORES, 2], F32)
            nc.sync.dma_start(out=ga2[:], in_=gat2[:].rearrange("(c p) s -> p c s", p=OUT))

            with nc.named_scope("combine2"):
                q2 = cst.tile([OUT, N_CORES, 2], F32)
                nc.vector.tensor_tensor(q2[:], ga2[:], ga2[:], MUL)
                msum2 = cst.tile([OUT, 2], F32)
                qsum2 = cst.tile([OUT, 2], F32)
                nc.vector.tensor_reduce(msum2[:], ga2[:].rearrange("p c s -> p s c"),
                                        mybir.AxisListType.X, ADD)
                nc.vector.tensor_reduce(qsum2[:], q2[:].rearrange("p c s -> p s c"),
                                        mybir.AxisListType.X, ADD)
                m2 = cst.tile([OUT, 1], F32)    # mean of raw mm2 (pre c2' shift)
                nc.vector.tensor_scalar(m2[:], msum2[:, 0:1], 1.0 / N_CORES, None, MUL)
                e22 = cst.tile([OUT, 1], F32)
                nc.vector.tensor_tensor(e22[:], qsum2[:, 0:1], msum2[:, 1:2], ADD)
                nc.vector.tensor_scalar(e22[:], e22[:], 1.0 / N_CORES, None, MUL)
                m2sq = cst.tile([OUT, 1], F32)
                nc.vector.tensor_tensor(m2sq[:], m2[:], m2[:], MUL)
                v2 = cst.tile([OUT, 1], F32)
                nc.vector.tensor_tensor(v2[:], e22[:], m2sq[:], SUB)
                sd2 = cst.tile([OUT, 1], F32)
                nc.vector.tensor_scalar(sd2[:], v2[:], 1.0, EPS, MUL, ADD)
                nc.scalar.sqrt(sd2[:], sd2[:])
                r2 = cst.tile([OUT, 1], F32)
                nc.vector.reciprocal(r2[:], sd2[:])
                scale2 = cst.tile([OUT, 1], F32)
                nc.vector.tensor_tensor(scale2[:], r2[:], g2c[:], MUL)
                # true h2 = mm2 - c2'; o = (mm2 - c2' - mu2_raw + c2')*scale2 + b2
                #        = (mm2 - m2)*scale2 + b2  (c2' cancels!)
                shift2 = cst.tile([OUT, 1], F32)
                nc.vector.tensor_tensor(shift2[:], m2[:], scale2[:], MUL)
                nc.vector.tensor_tensor(shift2[:], b2c[:], shift2[:], SUB)

            # final affine (in place), store transposed; host undoes the transpose
            for sl in range(4):
                ss = sl * (B_LOC // 4)
                se = ss + B_LOC // 4
                nc.vector.tensor_scalar(h2T[:, ss:se], h2T[:, ss:se], scale2[:], shift2[:], MUL, ADD)
                nc.sync.dma_start(out=o_out.ap()[:, ss:se], in_=h2T[:, ss:se])

    nc.compile()
    _cache["nc"] = nc
    return nc


def kernel(x, W1, W2, g1, b1, g2, b2, _trace=False):
    nc = build()
    x = np.ascontiguousarray(np.asarray(x, dtype=np.float32))
    in_maps = []
    for c in range(N_CORES):
        in_maps.append({
            "x": x[c * B_LOC:(c + 1) * B_LOC],
            "w1": np.asarray(W1, np.float32),
            "w2": np.asarray(W2, np.float32),
            "g1": np.asarray(g1, np.float32),
            "b1": np.asarray(b1, np.float32),
            "g2": np.asarray(g2, np.float32),
            "b2": np.asarray(b2, np.float32),
        })
    res = bass_utils.run_bass_kernel_spmd(nc, in_maps, core_ids=list(range(N_CORES)),
                                          trace=_trace)
    out = np.concatenate([np.ascontiguousarray(r["o_out"].T) for r in res.results], axis=0)
    if _trace:
        kernel.last_results = res
    return out


# revision 34
# speedup vs baseline: 1.0069x; 1.0069x over previous
"""Trainium2 Bass kernel for the binarized 2-layer MLP (nn_FC_small).

Network (reference semantics):
    h  = sign(x) @ sign(W1).T            # [B, 512], B = 65536, in = 768
    h  = batchnorm(h, g1, b1)            # training-mode, full-batch stats
    h  = clip(h, -1, 1)                  # hardtanh (sign-preserving)
    o  = sign(h) @ sign(W2).T            # [B, 10]
    o  = batchnorm(o, g2, b2)

Key identities used:
  * sign(clip(z)) == sign(z), and with g>0:
    sign((h-mu)*r*g + b) == sign(h - T), T = mu - b/(r*g).
  * sign(v) = 2*(v>0) - 1 for v != 0 (exact-zero ties handled by ACT Sign
    where it matters; measured data margin makes them impossible anyway).
  * mm = (x>0) @ sign(W1).T  =>  h1 = 2*mm - c1; threshold/stats are
    affine-transformed so only mm is ever materialized (exact integers).

Sharding: data-parallel over the batch across 8 NeuronCores (8192 rows each).
BN statistics are combined with two tiny AllGathers.

Dataflow per core:
  gpsimd cast-DMA x (fp32->bf16) -> PE-transpose (bf16, ~88ns/tile) ->
  fused code-gen evac from PSUM (DVE is_gt {0,1} / ACT Sign {+-1}, absorbed
  into per-chunk stationary scaling) -> fp8 DoubleRow matmul vs sign(W1).T
  -> exact integer counts in PSUM -> f16 h1-store + bn_stats -> warm
  AllGather of stats -> per-column thresholds -> {0,1} s-codes (DVE) ->
  bf16 matmul vs 2*sign(W2).T -> bn_stats -> AllGather -> affine ->
  PE-transpose to natural [8192, 10] output (host just concatenates).
"""
import numpy as np

import concourse.bass as bass
import concourse.bacc as bacc
import concourse.tile as tile
import concourse.mybir as mybir
from concourse import bass_utils
from concourse.masks import make_identity

F32 = mybir.dt.float32
F16 = mybir.dt.float16
BF16 = mybir.dt.bfloat16
FP8 = mybir.dt.float8e4
GT = mybir.AluOpType.is_gt
MUL = mybir.AluOpType.mult
ADD = mybir.AluOpType.add
SUB = mybir.AluOpType.subtract
DR = mybir.MatmulPerfMode.DoubleRow

N_CORES = 8
B = 65536
IND, HID, OUT = 768, 512, 10
B_LOC = B // N_CORES          # 8192
BC = 512                      # batch chunk
NBC = B_LOC // BC             # 16
KC = IND // 128               # 6 k-chunks
HC = HID // 128               # 4 hid-chunks
EPS = 1e-5

_cache = {}


def build():
    if "nc" in _cache:
        return _cache["nc"]
    nc = bacc.Bacc("TRN2", target_bir_lowering=False, debug=False,
                   num_devices=N_CORES)
    x = nc.dram_tensor("x", [B_LOC, IND], F32, kind="ExternalInput")
    w1 = nc.dram_tensor("w1", [HID, IND], F32, kind="ExternalInput")
    w2 = nc.dram_tensor("w2", [OUT, HID], F32, kind="ExternalInput")
    g1 = nc.dram_tensor("g1", [HID], F32, kind="ExternalInput")
    b1 = nc.dram_tensor("b1", [HID], F32, kind="ExternalInput")
    g2 = nc.dram_tensor("g2", [OUT], F32, kind="ExternalInput")
    b2 = nc.dram_tensor("b2", [OUT], F32, kind="ExternalInput")
    o_out = nc.dram_tensor("o_out", [OUT, B_LOC], F32, kind="ExternalOutput")

    with tile.TileContext(nc) as tc:
        with (
            tc.tile_pool(name="cst", bufs=1) as cst,      # constants / persistents
            tc.tile_pool(name="stage", bufs=3) as stage,  # streaming tiles
            tc.tile_pool(name="psx", bufs=4, space="PSUM") as psx,
            tc.tile_pool(name="psmm", bufs=1, space="PSUM") as psmm,
            tc.tile_pool(name="dram", bufs=1, space="DRAM") as dpool,
        ):
            ident = cst.tile([128, 128], BF16)
            make_identity(nc, ident[:])
            wup_sb = cst.tile([128, 1], F32)
            bias_m1 = cst.tile([128, 1], F32)
            nc.vector.memset(bias_m1[:], -1.0)
            bias_m2 = cst.tile([128, 1], F32)
            nc.vector.memset(bias_m2[:], -2.0)

            # prefetch the first x chunk ahead of weight prep
            xc_first = stage.tile([128, 4, IND], BF16, tag="xc")
            nc.gpsimd.dma_start(
                out=xc_first[:],
                in_=x.ap()[0:BC].rearrange("(s p) f -> p s f", p=128))

            # ---------------- weight prep (one-time, tiny) ----------------
            # W1 natural [512, 768] -> [128, 4, 768] (partition = hid%128)
            w1c = stage.tile([128, 4, IND], BF16, tag="xc")
            nc.gpsimd.dma_start(out=w1c[:], in_=w1.ap().rearrange("(c p) f -> p c f", p=128))
            w1b = stage.tile([128, 4, IND], BF16, tag="xc")
            nc.vector.tensor_scalar(w1b[:], w1c[:], 0.0, None, GT)
            # transpose each [128 hid, 128 feat] block -> [feat, hid]
            w1sT = cst.tile([128, KC, HID], FP8)     # sign(W1).T in fp8 (+-1)
            for k in range(KC):
                pw = psmm.tile([128, HC, 128], BF16, tag=f"mm{k % 4}")
                for c in range(HC):
                    nc.tensor.transpose(pw[:, c, :], w1b[:, c, k * 128:(k + 1) * 128], ident[:])
                # k<3 pairs with {0,1} x-codes: w values +-2; k>=3 pairs with
                # +-1 x-codes: w values +-1.  (2b-1)*s for s in {1,2}.
                sc1 = 2.0 if k >= 3 else 4.0
                nc.scalar.activation(w1sT[:, k, :], pw[:].rearrange("p c f -> p (c f)"),
                                     mybir.ActivationFunctionType.Identity,
                                     bias=(bias_m1[:] if k >= 3 else bias_m2[:]), scale=sc1)

            # ---------------- persistent big buffers ----------------
            xT8 = cst.tile([128, KC, B_LOC], FP8, tag="big8")   # 48 KB/part
            h1s = cst.tile([128, HC, B_LOC], F16, tag="hbig")   # 64 KB/part
            st1 = cst.tile([128, HC, NBC * 6], F32)        # bn_stats accum
            h2T = cst.tile([OUT, B_LOC], F32, tag="hbig")  # reuses h1s slot
            st2 = cst.tile([OUT, NBC * 6], F32)

            # warm-up collective: pays ncfw cold-start during phase A.
            # high_priority pins it to the schedule start (the CC instruction
            # is only a doorbell; completion is waited on by consumers only).
            wloc = dpool.tile([128, 1], F32)
            wgat = dpool.tile([128 * N_CORES, 1], F32)
            with tc.high_priority():
                nc.vector.memset(wup_sb[:], 0.0)
                nc.sync.dma_start(out=wloc[:], in_=wup_sb[:])
                nc.gpsimd.collective_compute(
                    "AllGather", mybir.AluOpType.bypass,
                    ins=[wloc.opt()], outs=[wgat.opt()],
                    replica_groups=[list(range(N_CORES))])

            # ---------------- phase A: x -> codes -> mm1 -> h1 store ----------------
            BLK = 4
            with nc.named_scope("phaseA"):
                for blk in range(NBC // BLK):
                    for b4 in range(BLK):
                        bc = blk * BLK + b4
                        bs = bc * BC
                        if bc == 0:
                            xc = xc_first
                        else:
                            xc = stage.tile([128, 4, IND], BF16, tag="xc")
                            nc.gpsimd.dma_start(
                                out=xc[:],
                                in_=x.ap()[bs:bs + BC].rearrange("(s p) f -> p s f", p=128))
                        for k in range(KC):
                            pt = psx.tile([128, 4, 128], BF16, tag="pt")
                            for s in range(4):
                                nc.tensor.transpose(pt[:, s, :], xc[:, s, k * 128:(k + 1) * 128], ident[:])
                            ptv = pt[:].rearrange("p s f -> p (s f)")
                            if k < 3:
                                nc.vector.tensor_scalar(xT8[:, k, bs:bs + BC], ptv, 0.0, None, GT)
                            else:
                                nc.scalar.sign(xT8[:, k, bs:bs + BC], ptv)
                    for h in range(HC):
                        mps = []
                        for b4 in range(BLK):
                            bs = (blk * BLK + b4) * BC
                            mp = psmm.tile([128, BC], F32, tag=f"mm{b4}")
                            mps.append(mp)
                            for k2 in range(KC // 2):
                                nc.tensor.matmul(
                                    mp[:],
                                    w1sT[:, 2 * k2:2 * k2 + 2, h * 128:(h + 1) * 128],
                                    xT8[:, 2 * k2:2 * k2 + 2, bs:bs + BC],
                                    start=(k2 == 0), stop=(k2 == KC // 2 - 1),
                                    perf_mode=DR)
                        for b4 in range(BLK):
                            bc = blk * BLK + b4
                            bs = bc * BC
                            nc.scalar.copy(h1s[:, h, bs:bs + BC], mps[b4][:])
                            nc.vector.bn_stats(st1[:, h, bc * 6:(bc + 1) * 6], mps[b4][:])

            # W2 natural [10, 512]
            w2n = cst.tile([OUT, HID], F32)
            nc.sync.dma_start(out=w2n[:], in_=w2.ap())
            w2b = cst.tile([OUT, HID], BF16)
            nc.vector.tensor_scalar(w2b[:], w2n[:], 0.0, None, GT)
            # sign(W2).T as fp8 [128, 4, 10]; hid-chunks 0,1 scaled x1 (for +-1 s),
            # hid-chunks 2,3 scaled x2 (for {0,1} s via is_gt; fold 2x into weight)
            w2sT = cst.tile([128, HC, 16], BF16)
            nc.vector.memset(w2sT[:], 0.0)
            for c in range(HC):
                pw2 = psmm.tile([128, OUT], BF16, tag="mm1")
                nc.tensor.transpose(pw2[:], w2b[:, c * 128:(c + 1) * 128], ident[:OUT, :OUT])
                sc = 4.0
                nc.scalar.activation(w2sT[:, c, 0:OUT], pw2[:],
                                     mybir.ActivationFunctionType.Identity,
                                     bias=bias_m2[:], scale=sc)
            # per-partition copies of g/b vectors
            g1c = cst.tile([128, HC], F32)
            b1c = cst.tile([128, HC], F32)
            for c in range(HC):
                nc.sync.dma_start(out=g1c[:, c:c + 1], in_=g1.ap()[c * 128:(c + 1) * 128])
                nc.sync.dma_start(out=b1c[:, c:c + 1], in_=b1.ap()[c * 128:(c + 1) * 128])
            g2c = cst.tile([OUT, 1], F32)
            b2c = cst.tile([OUT, 1], F32)
            nc.sync.dma_start(out=g2c[:], in_=g2.ap())
            nc.sync.dma_start(out=b2c[:], in_=b2.ap())

            # local aggregate -> [128, 4, 2] (mean, var) of mm per hid col
            agg1 = cst.tile([128, HC, 2], F32)
            for h in range(HC):
                nc.vector.bn_aggr(agg1[:, h, :], st1[:, h, :].rearrange("p (n s) -> p n s", s=6))

            # ---------------- AllGather 1 ----------------
            loc1 = dpool.tile([128, HC * 2], F32)
            gat1 = dpool.tile([128 * N_CORES, HC * 2], F32)
            nc.gpsimd.dma_start(out=loc1[:], in_=agg1[:].rearrange("p c s -> p (c s)"))
            nc.gpsimd.collective_compute(
                "AllGather", mybir.AluOpType.bypass,
                ins=[loc1.opt()], outs=[gat1.opt()],
                replica_groups=[list(range(N_CORES))])
            ga1 = cst.tile([128, N_CORES, HC * 2], F32)
            nc.sync.dma_start(out=ga1[:], in_=gat1[:].rearrange("(c p) s -> p c s", p=128))

            # combine: mean_tot = avg(mean_c); var_tot = avg(var_c + mean_c^2) - mean_tot^2
            with nc.named_scope("combine1"):
                q1 = cst.tile([128, N_CORES, HC * 2], F32)
                nc.vector.tensor_tensor(q1[:], ga1[:], ga1[:], MUL)
                msum = cst.tile([128, HC * 2], F32)
                qsum = cst.tile([128, HC * 2], F32)
                # reduce over the core axis in one op (view [p, s, c], reduce X)
                nc.vector.tensor_reduce(msum[:], ga1[:].rearrange("p c s -> p s c"),
                                        mybir.AxisListType.X, ADD)
                nc.vector.tensor_reduce(qsum[:], q1[:].rearrange("p c s -> p s c"),
                                        mybir.AxisListType.X, ADD)
                # views: even cols = means, odd = vars
                m1 = cst.tile([128, HC], F32)   # global mean of mm
                v1 = cst.tile([128, HC], F32)   # global var of mm
                mview = msum[:].rearrange("p (c s) -> p c s", s=2)
                qview = qsum[:].rearrange("p (c s) -> p c s", s=2)
                nc.vector.tensor_scalar(m1[:], mview[:, :, 0], 1.0 / N_CORES, None, MUL)
                # E[mm^2] = (sum var_c + sum mean_c^2)/8 ; var = E[mm^2] - m1^2
                e2 = cst.tile([128, HC], F32)
                nc.vector.tensor_tensor(e2[:], qview[:, :, 0], mview[:, :, 1], ADD)
                nc.vector.tensor_scalar(e2[:], e2[:], 1.0 / N_CORES, None, MUL)
                m1sq = cst.tile([128, HC], F32)
                nc.vector.tensor_tensor(m1sq[:], m1[:], m1[:], MUL)
                nc.vector.tensor_tensor(v1[:], e2[:], m1sq[:], SUB)
                # h1 = psum - c1_D (unit scale); sd1 = sqrt(v1 + eps) = 1/r1
                sd1 = cst.tile([128, HC], F32)
                nc.vector.tensor_scalar(sd1[:], v1[:], 1.0, EPS, MUL, ADD)
                nc.scalar.sqrt(sd1[:], sd1[:])
                # threshold in psum units: Tm = m1 - b1*sd1/g1
                ig1 = cst.tile([128, HC], F32)
                nc.vector.reciprocal(ig1[:], g1c[:])
                corr = cst.tile([128, HC], F32)
                nc.vector.tensor_tensor(corr[:], b1c[:], ig1[:], MUL)
                nc.vector.tensor_tensor(corr[:], corr[:], sd1[:], MUL)
                posT = cst.tile([128, HC], F32)   # +Tm for is_gt
                negT = cst.tile([128, HC], F32)   # -Tm for ACT Sign bias
                nc.vector.tensor_tensor(posT[:], m1[:], corr[:], SUB)
                nc.vector.tensor_scalar(negT[:], posT[:], -1.0, None, MUL)

            # ---------------- phase B: sign -> mm2 -> h2 ----------------
            with nc.named_scope("phaseB"):
                s8 = cst.tile([128, HC, B_LOC], BF16, tag="bigs")
                SLAB = 2048
                for sl in range(B_LOC // SLAB):
                    ss = sl * SLAB
                    for h in range(HC):
                        nc.vector.tensor_scalar(s8[:, h, ss:ss + SLAB], h1s[:, h, ss:ss + SLAB],
                                                posT[:, h:h + 1], None, GT)
                for bc in range(NBC):
                    bs = bc * BC
                    mp2 = psmm.tile([16, BC], F32, tag=f"mm{bc % 4}")
                    for k in range(HC):
                        nc.tensor.matmul(
                            mp2[:],
                            w2sT[:, k, :],
                            s8[:, k, bs:bs + BC],
                            start=(k == 0), stop=(k == HC - 1))
                    nc.scalar.copy(h2T[:, bs:bs + BC], mp2[:OUT, :])
                    nc.vector.bn_stats(st2[:, bc * 6:(bc + 1) * 6], mp2[:OUT, :])

            agg2 = cst.tile([OUT, 2], F32)
            nc.vector.bn_aggr(agg2[:], st2[:].rearrange("p (n s) -> p n s", s=6))

            # ---------------- AllGather 2 ----------------
            loc2 = dpool.tile([OUT, 2], F32)
            gat2 = dpool.tile([OUT * N_CORES, 2], F32)
            nc.gpsimd.dma_start(out=loc2[:], in_=agg2[:])
            nc.gpsimd.collective_compute(
                "AllGather", mybir.AluOpType.bypass,
                ins=[loc2.opt()], outs=[gat2.opt()],
                replica_groups=[list(range(N_CORES))])
            ga2 = cst.tile([OUT, N_CORES, 2], F32)
            nc.sync.dma_start(out=ga2[:], in_=gat2[:].rearrange("(c p) s -> p c s", p=OUT))

            with nc.named_scope("combine2"):
                q2 = cst.tile([OUT, N_CORES, 2], F32)
                nc.vector.tensor_tensor(q2[:], ga2[:], ga2[:], MUL)
                msum2 = cst.tile([OUT, 2], F32)
                qsum2 = cst.tile([OUT, 2], F32)
                nc.vector.tensor_reduce(msum2[:], ga2[:].rearrange("p c s -> p s c"),
                                        mybir.AxisListType.X, ADD)
                nc.vector.tensor_reduce(qsum2[:], q2[:].rearrange("p c s -> p s c"),
                                        mybir.AxisListType.X, ADD)
                m2 = cst.tile([OUT, 1], F32)    # mean of raw mm2 (pre c2' shift)
                nc.vector.tensor_scalar(m2[:], msum2[:, 0:1], 1.0 / N_CORES, None, MUL)
                e22 = cst.tile([OUT, 1], F32)
                nc.vector.tensor_tensor(e22[:], qsum2[:, 0:1], msum2[:, 1:2], ADD)
                nc.vector.tensor_scalar(e22[:], e22[:], 1.0 / N_CORES, None, MUL)
                m2sq = cst.tile([OUT, 1], F32)
                nc.vector.tensor_tensor(m2sq[:], m2[:], m2[:], MUL)
                v2 = cst.tile([OUT, 1], F32)
                nc.vector.tensor_tensor(v2[:], e22[:], m2sq[:], SUB)
                sd2 = cst.tile([OUT, 1], F32)
                nc.vector.tensor_scalar(sd2[:], v2[:], 1.0, EPS, MUL, ADD)
                nc.scalar.sqrt(sd2[:], sd2[:])
                r2 = cst.tile([OUT, 1], F32)
                nc.vector.reciprocal(r2[:], sd2[:])
                scale2 = cst.tile([OUT, 1], F32)
                nc.vector.tensor_tensor(scale2[:], r2[:], g2c[:], MUL)
                # true h2 = mm2 - c2'; o = (mm2 - c2' - mu2_raw + c2')*scale2 + b2
                #        = (mm2 - m2)*scale2 + b2  (c2' cancels!)
                shift2 = cst.tile([OUT, 1], F32)
                nc.vector.tensor_tensor(shift2[:], m2[:], scale2[:], MUL)
                nc.vector.tensor_tensor(shift2[:], b2c[:], shift2[:], SUB)

            # final affine (in place), store transposed; host undoes the transpose
            for sl in range(4):
                ss = sl * (B_LOC // 4)
                se = ss + B_LOC // 4
                nc.vector.tensor_scalar(h2T[:, ss:se], h2T[:, ss:se], scale2[:], shift2[:], MUL, ADD)
                nc.sync.dma_start(out=o_out.ap()[:, ss:se], in_=h2T[:, ss:se])

    nc.compile()
    _cache["nc"] = nc
    return nc


def kernel(x, W1, W2, g1, b1, g2, b2, _trace=False):
    nc = build()
    x = np.ascontiguousarray(np.asarray(x, dtype=np.float32))
    in_maps = []
    for c in range(N_CORES):
        in_maps.append({
            "x": x[c * B_LOC:(c + 1) * B_LOC],
            "w1": np.asarray(W1, np.float32),
            "w2": np.asarray(W2, np.float32),
            "g1": np.asarray(g1, np.float32),
            "b1": np.asarray(b1, np.float32),
            "g2": np.asarray(g2, np.float32),
            "b2": np.asarray(b2, np.float32),
        })
    res = bass_utils.run_bass_kernel_spmd(nc, in_maps, core_ids=list(range(N_CORES)),
                                          trace=_trace)
    out = np.concatenate([np.ascontiguousarray(r["o_out"].T) for r in res.results], axis=0)
    if _trace:
        kernel.last_results = res
    return out
